# revision 22
# baseline (speedup 1.0000x reference)
"""Causal GQA attention (prefill) on 8 TRN2 NeuronCores.

Problem: B=2, S=2048, H=32 query heads, Hk=8 kv heads, D=128, f32 I/O.
Sharding: tensor-parallel over heads -- core c gets query heads [4c, 4c+4)
and kv head c. Attention is fully independent per head: no collectives.

Per-core kernel: 8 instances of causal attention, one per (batch, qhead),
processed as a software-pipelined stream of 32 (instance, superblock)
items. Engine budget drives the design (ScalarE exp and PE matmul are
the two near-saturated engines; the PE is strictly in-order so the
emission order IS the schedule):
  - Q and K are pre-cast to bf16 AND pre-transposed to [d, s] layout on
    the host, so Q^T/K^T stream straight from DRAM into SBUF with plain
    contiguous DMAs: zero PE transposes, zero DVE copies, half the DMA
    bytes of f32.
  - QK^T is computed per (query-superblock 512, key-block 128) into
    3-bank PSUM page tiles (pool of 2); exp runs on ScalarE as ONE
    activation per page (up to 3 key blocks = 1536 cols, multi-bank
    read) to amortize the ~0.35us fixed cost per activation. The 4
    ragged diagonal blocks are packed into 2.5 banks of one page and
    exp'd by a single 1280-col activation -- no garbage columns.
  - P^T tiles feed PV matmuls as stationary weights; V carries an
    appended ones-column so the softmax denominator accumulates in the
    same PSUM tile (column 128). The 4 PV output slabs (129 cols each)
    are packed 2-per-bank into 2 PSUM banks via first-write start /
    last-write stop flags.
  - Items are phase-shifted: item n's PV matmuls are woven between item
    n+1's QK groups as in-order filler, so page-recycle and exp-latency
    waits never idle the PE (which would also drop its p-state clock).
  - out = PV / denom via one batched DVE reciprocal + one broadcast
    multiply per item.
"""

import numpy as np
import ml_dtypes

import concourse.bass as bass
import concourse.tile as tile
from concourse import bacc, mybir
from concourse.bass import ts
from concourse.bass_utils import run_bass_kernel_spmd
from concourse.masks import make_upper_triangular

B = 2
S = 2048
H = 32
HK = 8
D = 128
NCORES = 8
GH = H // NCORES  # query heads per core (= group size here)
SCALE = 0.08838834764831845  # 1/sqrt(128)

F32 = mybir.dt.float32
BF16 = mybir.dt.bfloat16

NQB = S // 128  # 16 query/key blocks of 128
NSB = 4  # query superblocks of 512
PAGE = 3  # PSUM page size in banks; two pages rotate


def build_nc() -> bass.Bass:
    nc = bacc.Bacc(
        "TRN2", target_bir_lowering=False, debug=False, num_devices=NCORES
    )
    # host-staged layouts: qT [b, g, d, s], kT [b, d, s], v [b, s, d]
    q_d = nc.declare_dram_parameter("query", [B, GH, D, S], BF16, isOutput=False)
    k_d = nc.declare_dram_parameter("key", [B, D, S], BF16, isOutput=False)
    v_d = nc.declare_dram_parameter("value", [B, S, D], BF16, isOutput=False)
    o_d = nc.declare_dram_parameter("out", [B, S, GH, D], F32, isOutput=True)

    with tile.TileContext(nc) as tc:
        with (
            tc.tile_pool(name="consts", bufs=1) as consts,
            tc.tile_pool(name="pt", bufs=8) as pt_pool,
            tc.tile_pool(name="ptd", bufs=3) as ptd_pool,
            tc.tile_pool(name="osb", bufs=3) as osb_pool,
            tc.tile_pool(name="small", bufs=8) as small_pool,
            tc.tile_pool(name="psum", bufs=1, space="PSUM") as psum_pool,
        ):
            # mask[k, q] = 1 where q >= k (keep), 0 above -> kills k > q.
            mask = consts.tile([128, 128], BF16)
            make_upper_triangular(nc, mask, val=1.0, diag=True)

            kt_all = consts.tile([128, B, S], BF16)  # [d, b, k]
            qt_all = consts.tile([128, B * GH, S], BF16)  # [d, inst, q]
            v_ext = consts.tile([128, B, NQB, 132], BF16)  # [k, b, kblk, d+1]

            # PSUM: two 3-bank QK page tiles (pool) + 2-bank packed PV acc.
            # Each page is its own pool tile so dependency tracking works
            # at page granularity: QK of group g+1 must not serialize
            # behind the exp of group g.
            po = psum_pool.tile([128, 2, 512], F32)

            def next_page():
                return psum_pool.tile(
                    [128, PAGE, 512], F32, tag="page", bufs=2, name="pg"
                )

            # ---- startup loads, first-needed first, in 512-col chunks so
            # item (0, sq) unblocks as soon as its kt/qt slices land.
            nc.vector.memset(v_ext[:, :, :, 128:129], 1.0)
            for c0 in range(0, 1024, 512):
                nc.sync.dma_start(
                    out=kt_all[:, 0, c0 : c0 + 512], in_=k_d[0, :, c0 : c0 + 512]
                )
                nc.sync.dma_start(
                    out=qt_all[:, 0, c0 : c0 + 512], in_=q_d[0, 0, :, c0 : c0 + 512]
                )
            nc.sync.dma_start(
                out=v_ext[:, 0, 0:8, 0:128],
                in_=v_d[0, 0:1024, :].rearrange("(n p) d -> p n d", p=128),
            )
            for c0 in range(1024, 2048, 512):
                nc.sync.dma_start(
                    out=kt_all[:, 0, c0 : c0 + 512], in_=k_d[0, :, c0 : c0 + 512]
                )
                nc.sync.dma_start(
                    out=qt_all[:, 0, c0 : c0 + 512], in_=q_d[0, 0, :, c0 : c0 + 512]
                )
            nc.sync.dma_start(
                out=v_ext[:, 0, 8:16, 0:128],
                in_=v_d[0, 1024:2048, :].rearrange("(n p) d -> p n d", p=128),
            )
            nc.sync.dma_start(out=qt_all[:, 1, :], in_=q_d[0, 1, :, :])
            nc.sync.dma_start(out=kt_all[:, 1, :], in_=k_d[1, :, :])
            nc.sync.dma_start(
                out=v_ext[:, 1, :, 0:128],
                in_=v_d[1, :, :].rearrange("(n p) d -> p n d", p=128),
            )

            # pending = (pv_closures, finish_closure, cursor) of prev item
            pending = [None]

            def emit_pending_chunk(frac_done):
                if pending[0] is None:
                    return
                pvs, _fin, cursor = pending[0]
                tgt = int(len(pvs) * frac_done + 0.5)
                while cursor[0] < tgt:
                    pvs[cursor[0]]()
                    cursor[0] += 1

            def finish_pending():
                if pending[0] is None:
                    return
                pvs, fin, cursor = pending[0]
                while cursor[0] < len(pvs):
                    pvs[cursor[0]]()
                    cursor[0] += 1
                fin()
                pending[0] = None

            def phase_item(inst, sq):
                b, g = divmod(inst, GH)
                q0 = 512 * sq
                noff = 4 * sq
                kd = 4 * sq
                ngroups = (noff + PAGE - 1) // PAGE + 1

                totals = [noff + 1 + j for j in range(4)]
                bank_tot = [totals[0] + totals[1], totals[2] + totals[3]]
                bank_cnt = [0, 0]
                pvs = []

                def defer_pv(pt_ap, kk, j):
                    def run(pt_ap=pt_ap, kk=kk, j=j):
                        bk = j // 2
                        off = (j % 2) * 256
                        bank_cnt[bk] += 1
                        nc.tensor.matmul(
                            po[:, bk, off : off + 129],
                            lhsT=pt_ap,
                            rhs=v_ext[:, b, kk, 0:129],
                            start=(bank_cnt[bk] == 1),
                            stop=(bank_cnt[bk] == bank_tot[bk]),
                        )
                    pvs.append(run)

                gi = 0
                ki = 0
                while ki < noff:
                    n = min(PAGE, noff - ki)
                    pg = next_page()
                    for t in range(n):
                        nc.tensor.matmul(
                            pg[:, t, :],
                            lhsT=kt_all[:, b, ts(ki + t, 128)],
                            rhs=qt_all[:, inst, q0 : q0 + 512],
                            start=True,
                            stop=True,
                        )
                    pt = pt_pool.tile([128, PAGE, 512], BF16)
                    nc.scalar.activation(
                        pt[:, 0:n, :],
                        pg[:, 0:n, :],
                        mybir.ActivationFunctionType.Exp,
                        scale=SCALE,
                    )
                    for t in range(n):
                        for j in range(4):
                            defer_pv(pt[:, t, ts(j, 128)], ki + t, j)
                    ki += n
                    gi += 1
                    emit_pending_chunk(gi / ngroups)

                # --- diagonal group: 4 ragged blocks packed into 2.5
                # banks of one page, one 1280-col exp. Flat col mapping:
                # ki+0 -> 0:512, ki+1 -> 512:896, ki+3 -> 896:1024,
                # ki+2 -> 1024:1280.
                pg = next_page()
                nc.tensor.matmul(
                    pg[:, 0, :],
                    lhsT=kt_all[:, b, ts(kd, 128)],
                    rhs=qt_all[:, inst, q0 : q0 + 512],
                    start=True, stop=True,
                )
                nc.tensor.matmul(
                    pg[:, 1, 0:384],
                    lhsT=kt_all[:, b, ts(kd + 1, 128)],
                    rhs=qt_all[:, inst, q0 + 128 : q0 + 512],
                    start=True, stop=False,
                )
                nc.tensor.matmul(
                    pg[:, 1, 384:512],
                    lhsT=kt_all[:, b, ts(kd + 3, 128)],
                    rhs=qt_all[:, inst, q0 + 384 : q0 + 512],
                    start=False, stop=True,
                )
                nc.tensor.matmul(
                    pg[:, 2, 0:256],
                    lhsT=kt_all[:, b, ts(kd + 2, 128)],
                    rhs=qt_all[:, inst, q0 + 256 : q0 + 512],
                    start=True, stop=True,
                )
                ptd = ptd_pool.tile([128, 1280], BF16)
                pgb = pg[:, :, :]
                diag_in = bass.AP(
                    pgb.tensor, pgb.offset, [list(pgb.ap[0]), [1, 1280]]
                )
                nc.scalar.activation(
                    ptd,
                    diag_in,
                    mybir.ActivationFunctionType.Exp,
                    scale=SCALE,
                )
                for c0 in (0, 512, 896, 1024):
                    nc.vector.tensor_mul(
                        ptd[:, c0 : c0 + 128], ptd[:, c0 : c0 + 128], mask
                    )

                for j in range(4):
                    defer_pv(ptd[:, 128 * j : 128 * (j + 1)], kd, j)
                for j in range(1, 4):
                    defer_pv(ptd[:, 512 + 128 * (j - 1) : 512 + 128 * j], kd + 1, j)
                for j in range(2, 4):
                    defer_pv(ptd[:, 1024 + 128 * (j - 2) : 1024 + 128 * (j - 1)], kd + 2, j)
                defer_pv(ptd[:, 896:1024], kd + 3, 3)

                # --- finish previous item (its remaining PV + normalize)
                finish_pending()

                def finish():
                    # batched normalize: slab j sits at po offset j*256,
                    # denominator at col 128 of each slab. One reciprocal
                    # over all 4 denominators + one broadcast multiply.
                    o_sb = osb_pool.tile([128, 4, 128], F32)
                    recip = small_pool.tile([128, 4], F32)
                    base = po[:, :, :]
                    pp = list(base.ap[0])
                    den_ap = bass.AP(
                        base.tensor, base.offset + 128, [pp, [256, 4], [1, 1]]
                    )
                    pv_ap = bass.AP(
                        base.tensor, base.offset, [pp, [256, 4], [1, 128]]
                    )
                    nc.vector.reciprocal(recip, den_ap)
                    rb = recip[:, :]
                    rb_b = bass.AP(
                        rb.tensor, rb.offset, [list(rb.ap[0]), [1, 4], [0, 128]]
                    )
                    ob = o_sb[:, :, :]
                    ob3 = bass.AP(
                        ob.tensor, ob.offset, [list(ob.ap[0]), [128, 4], [1, 128]]
                    )
                    nc.vector.tensor_mul(ob3, pv_ap, rb_b)
                    nc.sync.dma_start(
                        out=o_d[b, q0 : q0 + 512, g, :].rearrange(
                            "(n p) d -> p n d", p=128
                        ),
                        in_=o_sb,
                    )

                pending[0] = (pvs, finish, [0])

            # qt for inst is loaded two instances ahead so QK never waits.
            # The last instance runs its superblocks big-to-small so the
            # pipeline tail drains the 10-matmul sq0 item, not sq3's 58.
            for inst in range(B * GH):
                if inst + 2 < B * GH:
                    bn, gn = divmod(inst + 2, GH)
                    nc.sync.dma_start(
                        out=qt_all[:, inst + 2, :], in_=q_d[bn, gn, :, :]
                    )
                sqs = range(NSB - 1, -1, -1) if inst == B * GH - 1 else range(NSB)
                for sq in sqs:
                    phase_item(inst, sq)
            finish_pending()

    nc.finalize()
    return nc


def _to_bf16(x):
    return np.asarray(x, dtype=np.float32).astype(ml_dtypes.bfloat16)


def make_in_maps(query, key, value):
    # host-side staging: bf16 cast + [d, s] transposes for Q and K
    qb = _to_bf16(query)  # [B, S, H, D]
    kb = _to_bf16(key)  # [B, S, HK, D]
    vb = _to_bf16(value)
    in_maps = []
    for c in range(NCORES):
        qt = np.ascontiguousarray(
            qb[:, :, GH * c : GH * (c + 1), :].transpose(0, 2, 3, 1)
        )  # [B, GH, D, S]
        kt = np.ascontiguousarray(kb[:, :, c, :].transpose(0, 2, 1))  # [B, D, S]
        vv = np.ascontiguousarray(vb[:, :, c, :])  # [B, S, D]
        in_maps.append({"query": qt, "key": kt, "value": vv})
    return in_maps


def kernel(query, key, value):
    nc = build_nc()
    res = run_bass_kernel_spmd(
        nc, make_in_maps(query, key, value), core_ids=list(range(NCORES))
    )
    outs = [np.asarray(res.results[c]["out"]) for c in range(NCORES)]
    return np.concatenate(outs, axis=2).astype(np.float32)


if __name__ == "__main__":
    rng = np.random.default_rng(0)
    q = rng.standard_normal((B, S, H, D), dtype=np.float32)
    k = rng.standard_normal((B, S, HK, D), dtype=np.float32)
    v = rng.standard_normal((B, S, HK, D), dtype=np.float32)
    out = kernel(q, k, v)
    print("out", out.shape, out.dtype, float(np.abs(out).max()))


# revision 23
# speedup vs baseline: 1.0133x; 1.0133x over previous
"""Causal GQA attention (prefill) on 8 TRN2 NeuronCores.

Problem: B=2, S=2048, H=32 query heads, Hk=8 kv heads, D=128, f32 I/O.
Sharding: tensor-parallel over heads -- core c gets query heads [4c, 4c+4)
and kv head c. Attention is fully independent per head: no collectives.

Per-core kernel: 8 instances of causal attention, one per (batch, qhead),
processed as a software-pipelined stream of 32 (instance, superblock)
items. Engine budget drives the design (ScalarE exp and PE matmul are
the two near-saturated engines; the PE is strictly in-order so the
emission order IS the schedule):
  - Q and K are pre-cast to bf16 AND pre-transposed to [d, s] layout on
    the host, so Q^T/K^T stream straight from DRAM into SBUF with plain
    contiguous DMAs: zero PE transposes, zero DVE copies, half the DMA
    bytes of f32.
  - QK^T is computed per (query-superblock 512, key-block 128) into
    3-bank PSUM page tiles (pool of 2); exp runs on ScalarE as ONE
    activation per page (up to 3 key blocks = 1536 cols, multi-bank
    read) to amortize the ~0.35us fixed cost per activation. The 4
    ragged diagonal blocks are packed into 2.5 banks of one page and
    exp'd by a single 1280-col activation -- no garbage columns.
  - P^T tiles feed PV matmuls as stationary weights; V carries an
    appended ones-column so the softmax denominator accumulates in the
    same PSUM tile (column 128). The 4 PV output slabs (129 cols each)
    are packed 2-per-bank into 2 PSUM banks via first-write start /
    last-write stop flags.
  - Items are phase-shifted: item n's PV matmuls are woven between item
    n+1's QK groups as in-order filler, so page-recycle and exp-latency
    waits never idle the PE (which would also drop its p-state clock).
  - out = PV / denom via one batched DVE reciprocal + one broadcast
    multiply per item.
"""

import numpy as np
import ml_dtypes

import concourse.bass as bass
import concourse.tile as tile
from concourse import bacc, mybir
from concourse.bass import ts
from concourse.bass_utils import run_bass_kernel_spmd
from concourse.masks import make_upper_triangular

B = 2
S = 2048
H = 32
HK = 8
D = 128
NCORES = 8
GH = H // NCORES  # query heads per core (= group size here)
SCALE = 0.08838834764831845  # 1/sqrt(128)

F32 = mybir.dt.float32
BF16 = mybir.dt.bfloat16

NQB = S // 128  # 16 query/key blocks of 128
NSB = 4  # query superblocks of 512
PAGE = 3  # PSUM page size in banks; two pages rotate


def build_nc() -> bass.Bass:
    nc = bacc.Bacc(
        "TRN2", target_bir_lowering=False, debug=False, num_devices=NCORES
    )
    # host-staged layouts: qT [b, g, d, s], kT [b, d, s], v [b, s, d]
    q_d = nc.declare_dram_parameter("query", [B, GH, D, S], BF16, isOutput=False)
    k_d = nc.declare_dram_parameter("key", [B, D, S], BF16, isOutput=False)
    v_d = nc.declare_dram_parameter("value", [B, S, D], BF16, isOutput=False)
    o_d = nc.declare_dram_parameter("out", [B, S, GH, D], F32, isOutput=True)

    with tile.TileContext(nc) as tc:
        with (
            tc.tile_pool(name="consts", bufs=1) as consts,
            tc.tile_pool(name="pt", bufs=8) as pt_pool,
            tc.tile_pool(name="ptd", bufs=3) as ptd_pool,
            tc.tile_pool(name="osb", bufs=3) as osb_pool,
            tc.tile_pool(name="small", bufs=8) as small_pool,
            tc.tile_pool(name="psum", bufs=1, space="PSUM") as psum_pool,
        ):
            # mask[k, q] = 1 where q >= k (keep), 0 above -> kills k > q.
            mask = consts.tile([128, 128], BF16)
            make_upper_triangular(nc, mask, val=1.0, diag=True)

            kt_all = consts.tile([128, B, S], BF16)  # [d, b, k]
            qt_all = consts.tile([128, B * GH, S], BF16)  # [d, inst, q]
            v_ext = consts.tile([128, B, NQB, 132], BF16)  # [k, b, kblk, d+1]

            # PSUM: two 3-bank QK page tiles (pool) + 2-bank packed PV acc.
            # Each page is its own pool tile so dependency tracking works
            # at page granularity: QK of group g+1 must not serialize
            # behind the exp of group g.
            po = psum_pool.tile([128, 2, 512], F32)

            def next_page():
                return psum_pool.tile(
                    [128, PAGE, 512], F32, tag="page", bufs=2, name="pg"
                )

            # ---- startup loads, first-needed first, in 512-col chunks so
            # item (0, sq) unblocks as soon as its kt/qt slices land.
            nc.vector.memset(v_ext[:, :, :, 128:129], 1.0)
            for c0 in range(0, 1024, 512):
                nc.sync.dma_start(
                    out=kt_all[:, 0, c0 : c0 + 512], in_=k_d[0, :, c0 : c0 + 512]
                )
                nc.sync.dma_start(
                    out=qt_all[:, 0, c0 : c0 + 512], in_=q_d[0, 0, :, c0 : c0 + 512]
                )
            nc.sync.dma_start(
                out=v_ext[:, 0, 0:8, 0:128],
                in_=v_d[0, 0:1024, :].rearrange("(n p) d -> p n d", p=128),
            )
            for c0 in range(1024, 2048, 512):
                nc.sync.dma_start(
                    out=kt_all[:, 0, c0 : c0 + 512], in_=k_d[0, :, c0 : c0 + 512]
                )
                nc.sync.dma_start(
                    out=qt_all[:, 0, c0 : c0 + 512], in_=q_d[0, 0, :, c0 : c0 + 512]
                )
            nc.sync.dma_start(
                out=v_ext[:, 0, 8:16, 0:128],
                in_=v_d[0, 1024:2048, :].rearrange("(n p) d -> p n d", p=128),
            )
            nc.sync.dma_start(out=qt_all[:, 1, :], in_=q_d[0, 1, :, :])
            nc.sync.dma_start(out=kt_all[:, 1, :], in_=k_d[1, :, :])
            nc.sync.dma_start(
                out=v_ext[:, 1, :, 0:128],
                in_=v_d[1, :, :].rearrange("(n p) d -> p n d", p=128),
            )

            # pending = (pv_closures, finish_closure, cursor) of prev item
            pending = [None]

            def emit_pending_chunk(frac_done):
                if pending[0] is None:
                    return
                pvs, _fin, cursor = pending[0]
                tgt = int(len(pvs) * frac_done + 0.5)
                while cursor[0] < tgt:
                    pvs[cursor[0]]()
                    cursor[0] += 1

            def finish_pending():
                if pending[0] is None:
                    return
                pvs, fin, cursor = pending[0]
                while cursor[0] < len(pvs):
                    pvs[cursor[0]]()
                    cursor[0] += 1
                fin()
                pending[0] = None

            def phase_item(inst, sq):
                b, g = divmod(inst, GH)
                q0 = 512 * sq
                noff = 4 * sq
                kd = 4 * sq
                ngroups = (noff + PAGE - 1) // PAGE + 1

                totals = [noff + 1 + j for j in range(4)]
                bank_tot = [totals[0] + totals[1], totals[2] + totals[3]]
                bank_cnt = [0, 0]
                pvs = []

                def defer_pv(pt_ap, kk, j):
                    def run(pt_ap=pt_ap, kk=kk, j=j):
                        bk = j // 2
                        off = (j % 2) * 256
                        bank_cnt[bk] += 1
                        nc.tensor.matmul(
                            po[:, bk, off : off + 129],
                            lhsT=pt_ap,
                            rhs=v_ext[:, b, kk, 0:129],
                            start=(bank_cnt[bk] == 1),
                            stop=(bank_cnt[bk] == bank_tot[bk]),
                        )
                    pvs.append(run)

                gi = 0
                ki = 0
                while ki < noff:
                    n = min(PAGE, noff - ki)
                    pg = next_page()
                    for t in range(n):
                        nc.tensor.matmul(
                            pg[:, t, :],
                            lhsT=kt_all[:, b, ts(ki + t, 128)],
                            rhs=qt_all[:, inst, q0 : q0 + 512],
                            start=True,
                            stop=True,
                        )
                    pt = pt_pool.tile([128, PAGE, 512], BF16)
                    nc.scalar.activation(
                        pt[:, 0:n, :],
                        pg[:, 0:n, :],
                        mybir.ActivationFunctionType.Exp,
                        scale=SCALE,
                    )
                    for t in range(n):
                        for j in range(4):
                            defer_pv(pt[:, t, ts(j, 128)], ki + t, j)
                    ki += n
                    gi += 1
                    emit_pending_chunk(gi / ngroups)

                # --- diagonal group: 4 ragged blocks packed into 2.5
                # banks of one page, one 1280-col exp. Flat col mapping:
                # ki+0 -> 0:512, ki+1 -> 512:896, ki+3 -> 896:1024,
                # ki+2 -> 1024:1280.
                pg = next_page()
                nc.tensor.matmul(
                    pg[:, 0, :],
                    lhsT=kt_all[:, b, ts(kd, 128)],
                    rhs=qt_all[:, inst, q0 : q0 + 512],
                    start=True, stop=True,
                )
                nc.tensor.matmul(
                    pg[:, 1, 0:384],
                    lhsT=kt_all[:, b, ts(kd + 1, 128)],
                    rhs=qt_all[:, inst, q0 + 128 : q0 + 512],
                    start=True, stop=False,
                )
                nc.tensor.matmul(
                    pg[:, 1, 384:512],
                    lhsT=kt_all[:, b, ts(kd + 3, 128)],
                    rhs=qt_all[:, inst, q0 + 384 : q0 + 512],
                    start=False, stop=True,
                )
                nc.tensor.matmul(
                    pg[:, 2, 0:256],
                    lhsT=kt_all[:, b, ts(kd + 2, 128)],
                    rhs=qt_all[:, inst, q0 + 256 : q0 + 512],
                    start=True, stop=True,
                )
                ptd = ptd_pool.tile([128, 1280], BF16)
                pgb = pg[:, :, :]
                diag_in = bass.AP(
                    pgb.tensor, pgb.offset, [list(pgb.ap[0]), [1, 1280]]
                )
                nc.scalar.activation(
                    ptd,
                    diag_in,
                    mybir.ActivationFunctionType.Exp,
                    scale=SCALE,
                )
                for c0 in (0, 512, 896, 1024):
                    nc.vector.tensor_mul(
                        ptd[:, c0 : c0 + 128], ptd[:, c0 : c0 + 128], mask
                    )

                for j in range(4):
                    defer_pv(ptd[:, 128 * j : 128 * (j + 1)], kd, j)
                for j in range(1, 4):
                    defer_pv(ptd[:, 512 + 128 * (j - 1) : 512 + 128 * j], kd + 1, j)
                for j in range(2, 4):
                    defer_pv(ptd[:, 1024 + 128 * (j - 2) : 1024 + 128 * (j - 1)], kd + 2, j)
                defer_pv(ptd[:, 896:1024], kd + 3, 3)

                # --- finish previous item (its remaining PV + normalize)
                finish_pending()

                def finish():
                    # batched normalize: slab j sits at po offset j*256,
                    # denominator at col 128 of each slab. One reciprocal
                    # over all 4 denominators + one broadcast multiply.
                    o_sb = osb_pool.tile([128, 4, 128], F32)
                    recip = small_pool.tile([128, 4], F32)
                    base = po[:, :, :]
                    pp = list(base.ap[0])
                    den_ap = bass.AP(
                        base.tensor, base.offset + 128, [pp, [256, 4], [1, 1]]
                    )
                    pv_ap = bass.AP(
                        base.tensor, base.offset, [pp, [256, 4], [1, 128]]
                    )
                    nc.vector.reciprocal(recip, den_ap)
                    rb = recip[:, :]
                    rb_b = bass.AP(
                        rb.tensor, rb.offset, [list(rb.ap[0]), [1, 4], [0, 128]]
                    )
                    ob = o_sb[:, :, :]
                    ob3 = bass.AP(
                        ob.tensor, ob.offset, [list(ob.ap[0]), [128, 4], [1, 128]]
                    )
                    nc.vector.tensor_mul(ob3, pv_ap, rb_b)
                    nc.sync.dma_start(
                        out=o_d[b, q0 : q0 + 512, g, :].rearrange(
                            "(n p) d -> p n d", p=128
                        ),
                        in_=o_sb,
                    )

                pending[0] = (pvs, finish, [0])

            # qt for inst is loaded two instances ahead so QK never waits.
            # The last instance runs its superblocks big-to-small so the
            # pipeline tail drains the 10-matmul sq0 item, not sq3's 58.
            for inst in range(B * GH):
                if inst + 2 < B * GH:
                    bn, gn = divmod(inst + 2, GH)
                    nc.sync.dma_start(
                        out=qt_all[:, inst + 2, :], in_=q_d[bn, gn, :, :]
                    )
                for sq in range(NSB):
                    phase_item(inst, sq)
            finish_pending()

    nc.finalize()
    return nc


def _to_bf16(x):
    return np.asarray(x, dtype=np.float32).astype(ml_dtypes.bfloat16)


def make_in_maps(query, key, value):
    # host-side staging: bf16 cast + [d, s] transposes for Q and K
    qb = _to_bf16(query)  # [B, S, H, D]
    kb = _to_bf16(key)  # [B, S, HK, D]
    vb = _to_bf16(value)
    in_maps = []
    for c in range(NCORES):
        qt = np.ascontiguousarray(
            qb[:, :, GH * c : GH * (c + 1), :].transpose(0, 2, 3, 1)
        )  # [B, GH, D, S]
        kt = np.ascontiguousarray(kb[:, :, c, :].transpose(0, 2, 1))  # [B, D, S]
        vv = np.ascontiguousarray(vb[:, :, c, :])  # [B, S, D]
        in_maps.append({"query": qt, "key": kt, "value": vv})
    return in_maps


def kernel(query, key, value):
    nc = build_nc()
    res = run_bass_kernel_spmd(
        nc, make_in_maps(query, key, value), core_ids=list(range(NCORES))
    )
    outs = [np.asarray(res.results[c]["out"]) for c in range(NCORES)]
    return np.concatenate(outs, axis=2).astype(np.float32)


if __name__ == "__main__":
    rng = np.random.default_rng(0)
    q = rng.standard_normal((B, S, H, D), dtype=np.float32)
    k = rng.standard_normal((B, S, HK, D), dtype=np.float32)
    v = rng.standard_normal((B, S, HK, D), dtype=np.float32)
    out = kernel(q, k, v)
    print("out", out.shape, out.dtype, float(np.abs(out).max()))


# revision 24
# speedup vs baseline: 1.0140x; 1.0006x over previous
"""Causal GQA attention (prefill) on 8 TRN2 NeuronCores.

Problem: B=2, S=2048, H=32 query heads, Hk=8 kv heads, D=128, f32 I/O.
Sharding: tensor-parallel over heads -- core c gets query heads [4c, 4c+4)
and kv head c. Attention is fully independent per head: no collectives.

Per-core kernel: 8 instances of causal attention, one per (batch, qhead),
processed as a software-pipelined stream of 32 (instance, superblock)
items. Engine budget drives the design (ScalarE exp and PE matmul are
the two near-saturated engines; the PE is strictly in-order so the
emission order IS the schedule):
  - Q and K are pre-cast to bf16 AND pre-transposed to [d, s] layout on
    the host, so Q^T/K^T stream straight from DRAM into SBUF with plain
    contiguous DMAs: zero PE transposes, zero DVE copies, half the DMA
    bytes of f32.
  - QK^T is computed per (query-superblock 512, key-block 128) into
    3-bank PSUM page tiles (pool of 2); exp runs on ScalarE as ONE
    activation per page (up to 3 key blocks = 1536 cols, multi-bank
    read) to amortize the ~0.35us fixed cost per activation. The 4
    ragged diagonal blocks are packed into 2.5 banks of one page and
    exp'd by a single 1280-col activation -- no garbage columns.
  - P^T tiles feed PV matmuls as stationary weights; V carries an
    appended ones-column so the softmax denominator accumulates in the
    same PSUM tile (column 128). The 4 PV output slabs (129 cols each)
    are packed 2-per-bank into 2 PSUM banks via first-write start /
    last-write stop flags.
  - Items are phase-shifted: item n's PV matmuls are woven between item
    n+1's QK groups as in-order filler, so page-recycle and exp-latency
    waits never idle the PE (which would also drop its p-state clock).
  - out = PV / denom via one batched DVE reciprocal + one broadcast
    multiply per item.
"""

import numpy as np
import ml_dtypes

import concourse.bass as bass
import concourse.tile as tile
from concourse import bacc, mybir
from concourse.bass import ts
from concourse.bass_utils import run_bass_kernel_spmd
from concourse.masks import make_upper_triangular

B = 2
S = 2048
H = 32
HK = 8
D = 128
NCORES = 8
GH = H // NCORES  # query heads per core (= group size here)
SCALE = 0.08838834764831845  # 1/sqrt(128)

F32 = mybir.dt.float32
BF16 = mybir.dt.bfloat16

NQB = S // 128  # 16 query/key blocks of 128
NSB = 4  # query superblocks of 512
PAGE = 3  # PSUM page size in banks; two pages rotate


def build_nc() -> bass.Bass:
    nc = bacc.Bacc(
        "TRN2", target_bir_lowering=False, debug=False, num_devices=NCORES
    )
    # host-staged layouts: qT [b, g, d, s], kT [b, d, s], v [b, s, d]
    q_d = nc.declare_dram_parameter("query", [B, GH, D, S], BF16, isOutput=False)
    k_d = nc.declare_dram_parameter("key", [B, D, S], BF16, isOutput=False)
    v_d = nc.declare_dram_parameter("value", [B, S, D], BF16, isOutput=False)
    o_d = nc.declare_dram_parameter("out", [B, S, GH, D], F32, isOutput=True)

    with tile.TileContext(nc) as tc:
        with (
            tc.tile_pool(name="consts", bufs=1) as consts,
            tc.tile_pool(name="pt", bufs=11) as pt_pool,
            tc.tile_pool(name="ptd", bufs=4) as ptd_pool,
            tc.tile_pool(name="osb", bufs=3) as osb_pool,
            tc.tile_pool(name="small", bufs=8) as small_pool,
            tc.tile_pool(name="psum", bufs=1, space="PSUM") as psum_pool,
        ):
            # mask[k, q] = 1 where q >= k (keep), 0 above -> kills k > q.
            mask = consts.tile([128, 128], BF16)
            make_upper_triangular(nc, mask, val=1.0, diag=True)

            kt_all = consts.tile([128, B, S], BF16)  # [d, b, k]
            qt_all = consts.tile([128, B * GH, S], BF16)  # [d, inst, q]
            v_ext = consts.tile([128, B, NQB, 132], BF16)  # [k, b, kblk, d+1]

            # PSUM: two 3-bank QK page tiles (pool) + 2-bank packed PV acc.
            # Each page is its own pool tile so dependency tracking works
            # at page granularity: QK of group g+1 must not serialize
            # behind the exp of group g.
            po = psum_pool.tile([128, 2, 512], F32)

            def next_page():
                return psum_pool.tile(
                    [128, PAGE, 512], F32, tag="page", bufs=2, name="pg"
                )

            # ---- startup loads, first-needed first, in 512-col chunks so
            # item (0, sq) unblocks as soon as its kt/qt slices land.
            nc.vector.memset(v_ext[:, :, :, 128:129], 1.0)
            for c0 in range(0, 1024, 512):
                nc.sync.dma_start(
                    out=kt_all[:, 0, c0 : c0 + 512], in_=k_d[0, :, c0 : c0 + 512]
                )
                nc.sync.dma_start(
                    out=qt_all[:, 0, c0 : c0 + 512], in_=q_d[0, 0, :, c0 : c0 + 512]
                )
            nc.sync.dma_start(
                out=v_ext[:, 0, 0:8, 0:128],
                in_=v_d[0, 0:1024, :].rearrange("(n p) d -> p n d", p=128),
            )
            for c0 in range(1024, 2048, 512):
                nc.sync.dma_start(
                    out=kt_all[:, 0, c0 : c0 + 512], in_=k_d[0, :, c0 : c0 + 512]
                )
                nc.sync.dma_start(
                    out=qt_all[:, 0, c0 : c0 + 512], in_=q_d[0, 0, :, c0 : c0 + 512]
                )
            nc.sync.dma_start(
                out=v_ext[:, 0, 8:16, 0:128],
                in_=v_d[0, 1024:2048, :].rearrange("(n p) d -> p n d", p=128),
            )
            nc.sync.dma_start(out=qt_all[:, 1, :], in_=q_d[0, 1, :, :])
            nc.sync.dma_start(out=kt_all[:, 1, :], in_=k_d[1, :, :])
            nc.sync.dma_start(
                out=v_ext[:, 1, :, 0:128],
                in_=v_d[1, :, :].rearrange("(n p) d -> p n d", p=128),
            )

            # pending = (pv_closures, finish_closure, cursor) of prev item
            pending = [None]

            def emit_pending_chunk(frac_done):
                if pending[0] is None:
                    return
                pvs, _fin, cursor = pending[0]
                tgt = int(len(pvs) * frac_done + 0.5)
                while cursor[0] < tgt:
                    pvs[cursor[0]]()
                    cursor[0] += 1

            def finish_pending():
                if pending[0] is None:
                    return
                pvs, fin, cursor = pending[0]
                while cursor[0] < len(pvs):
                    pvs[cursor[0]]()
                    cursor[0] += 1
                fin()
                pending[0] = None

            def phase_item(inst, sq):
                b, g = divmod(inst, GH)
                q0 = 512 * sq
                noff = 4 * sq
                kd = 4 * sq
                ngroups = (noff + PAGE - 1) // PAGE + 1

                totals = [noff + 1 + j for j in range(4)]
                bank_tot = [totals[0] + totals[1], totals[2] + totals[3]]
                bank_cnt = [0, 0]
                pvs = []

                def defer_pv(pt_ap, kk, j):
                    def run(pt_ap=pt_ap, kk=kk, j=j):
                        bk = j // 2
                        off = (j % 2) * 256
                        bank_cnt[bk] += 1
                        nc.tensor.matmul(
                            po[:, bk, off : off + 129],
                            lhsT=pt_ap,
                            rhs=v_ext[:, b, kk, 0:129],
                            start=(bank_cnt[bk] == 1),
                            stop=(bank_cnt[bk] == bank_tot[bk]),
                        )
                    pvs.append(run)

                gi = 0
                ki = 0
                while ki < noff:
                    n = min(PAGE, noff - ki)
                    pg = next_page()
                    for t in range(n):
                        nc.tensor.matmul(
                            pg[:, t, :],
                            lhsT=kt_all[:, b, ts(ki + t, 128)],
                            rhs=qt_all[:, inst, q0 : q0 + 512],
                            start=True,
                            stop=True,
                        )
                    pt = pt_pool.tile([128, PAGE, 512], BF16)
                    nc.scalar.activation(
                        pt[:, 0:n, :],
                        pg[:, 0:n, :],
                        mybir.ActivationFunctionType.Exp,
                        scale=SCALE,
                    )
                    for t in range(n):
                        for j in range(4):
                            defer_pv(pt[:, t, ts(j, 128)], ki + t, j)
                    ki += n
                    gi += 1
                    emit_pending_chunk(gi / ngroups)

                # --- diagonal group: 4 ragged blocks packed into 2.5
                # banks of one page, one 1280-col exp. Flat col mapping:
                # ki+0 -> 0:512, ki+1 -> 512:896, ki+3 -> 896:1024,
                # ki+2 -> 1024:1280.
                pg = next_page()
                nc.tensor.matmul(
                    pg[:, 0, :],
                    lhsT=kt_all[:, b, ts(kd, 128)],
                    rhs=qt_all[:, inst, q0 : q0 + 512],
                    start=True, stop=True,
                )
                nc.tensor.matmul(
                    pg[:, 1, 0:384],
                    lhsT=kt_all[:, b, ts(kd + 1, 128)],
                    rhs=qt_all[:, inst, q0 + 128 : q0 + 512],
                    start=True, stop=False,
                )
                nc.tensor.matmul(
                    pg[:, 1, 384:512],
                    lhsT=kt_all[:, b, ts(kd + 3, 128)],
                    rhs=qt_all[:, inst, q0 + 384 : q0 + 512],
                    start=False, stop=True,
                )
                nc.tensor.matmul(
                    pg[:, 2, 0:256],
                    lhsT=kt_all[:, b, ts(kd + 2, 128)],
                    rhs=qt_all[:, inst, q0 + 256 : q0 + 512],
                    start=True, stop=True,
                )
                ptd = ptd_pool.tile([128, 1280], BF16)
                pgb = pg[:, :, :]
                diag_in = bass.AP(
                    pgb.tensor, pgb.offset, [list(pgb.ap[0]), [1, 1280]]
                )
                nc.scalar.activation(
                    ptd,
                    diag_in,
                    mybir.ActivationFunctionType.Exp,
                    scale=SCALE,
                )
                for c0 in (0, 512, 896, 1024):
                    nc.vector.tensor_mul(
                        ptd[:, c0 : c0 + 128], ptd[:, c0 : c0 + 128], mask
                    )

                for j in range(4):
                    defer_pv(ptd[:, 128 * j : 128 * (j + 1)], kd, j)
                for j in range(1, 4):
                    defer_pv(ptd[:, 512 + 128 * (j - 1) : 512 + 128 * j], kd + 1, j)
                for j in range(2, 4):
                    defer_pv(ptd[:, 1024 + 128 * (j - 2) : 1024 + 128 * (j - 1)], kd + 2, j)
                defer_pv(ptd[:, 896:1024], kd + 3, 3)

                # --- finish previous item (its remaining PV + normalize)
                finish_pending()

                def finish():
                    # batched normalize: slab j sits at po offset j*256,
                    # denominator at col 128 of each slab. One reciprocal
                    # over all 4 denominators + one broadcast multiply.
                    o_sb = osb_pool.tile([128, 4, 128], F32)
                    recip = small_pool.tile([128, 4], F32)
                    base = po[:, :, :]
                    pp = list(base.ap[0])
                    den_ap = bass.AP(
                        base.tensor, base.offset + 128, [pp, [256, 4], [1, 1]]
                    )
                    pv_ap = bass.AP(
                        base.tensor, base.offset, [pp, [256, 4], [1, 128]]
                    )
                    nc.vector.reciprocal(recip, den_ap)
                    rb = recip[:, :]
                    rb_b = bass.AP(
                        rb.tensor, rb.offset, [list(rb.ap[0]), [1, 4], [0, 128]]
                    )
                    ob = o_sb[:, :, :]
                    ob3 = bass.AP(
                        ob.tensor, ob.offset, [list(ob.ap[0]), [128, 4], [1, 128]]
                    )
                    nc.vector.tensor_mul(ob3, pv_ap, rb_b)
                    nc.sync.dma_start(
                        out=o_d[b, q0 : q0 + 512, g, :].rearrange(
                            "(n p) d -> p n d", p=128
                        ),
                        in_=o_sb,
                    )

                pending[0] = (pvs, finish, [0])

            # qt for inst is loaded two instances ahead so QK never waits.
            # The last instance runs its superblocks big-to-small so the
            # pipeline tail drains the 10-matmul sq0 item, not sq3's 58.
            for inst in range(B * GH):
                if inst + 2 < B * GH:
                    bn, gn = divmod(inst + 2, GH)
                    nc.sync.dma_start(
                        out=qt_all[:, inst + 2, :], in_=q_d[bn, gn, :, :]
                    )
                for sq in range(NSB):
                    phase_item(inst, sq)
            finish_pending()

    nc.finalize()
    return nc


def _to_bf16(x):
    return np.asarray(x, dtype=np.float32).astype(ml_dtypes.bfloat16)


def make_in_maps(query, key, value):
    # host-side staging: bf16 cast + [d, s] transposes for Q and K
    qb = _to_bf16(query)  # [B, S, H, D]
    kb = _to_bf16(key)  # [B, S, HK, D]
    vb = _to_bf16(value)
    in_maps = []
    for c in range(NCORES):
        qt = np.ascontiguousarray(
            qb[:, :, GH * c : GH * (c + 1), :].transpose(0, 2, 3, 1)
        )  # [B, GH, D, S]
        kt = np.ascontiguousarray(kb[:, :, c, :].transpose(0, 2, 1))  # [B, D, S]
        vv = np.ascontiguousarray(vb[:, :, c, :])  # [B, S, D]
        in_maps.append({"query": qt, "key": kt, "value": vv})
    return in_maps


def kernel(query, key, value):
    nc = build_nc()
    res = run_bass_kernel_spmd(
        nc, make_in_maps(query, key, value), core_ids=list(range(NCORES))
    )
    outs = [np.asarray(res.results[c]["out"]) for c in range(NCORES)]
    return np.concatenate(outs, axis=2).astype(np.float32)


if __name__ == "__main__":
    rng = np.random.default_rng(0)
    q = rng.standard_normal((B, S, H, D), dtype=np.float32)
    k = rng.standard_normal((B, S, HK, D), dtype=np.float32)
    v = rng.standard_normal((B, S, HK, D), dtype=np.float32)
    out = kernel(q, k, v)
    print("out", out.shape, out.dtype, float(np.abs(out).max()))


# revision 27
# speedup vs baseline: 1.0171x; 1.0031x over previous
"""Causal GQA attention (prefill) on 8 TRN2 NeuronCores.

Problem: B=2, S=2048, H=32 query heads, Hk=8 kv heads, D=128, f32 I/O.
Sharding: tensor-parallel over heads -- core c gets query heads [4c, 4c+4)
and kv head c. Attention is fully independent per head: no collectives.

Per-core kernel: 8 instances of causal attention, one per (batch, qhead),
processed as a software-pipelined stream of 32 (instance, superblock)
items. Engine budget drives the design (ScalarE exp and PE matmul are
the two near-saturated engines; the PE is strictly in-order so the
emission order IS the schedule):
  - Q and K are pre-cast to bf16 AND pre-transposed to [d, s] layout on
    the host, so Q^T/K^T stream straight from DRAM into SBUF with plain
    contiguous DMAs: zero PE transposes, zero DVE copies, half the DMA
    bytes of f32.
  - QK^T is computed per (query-superblock 512, key-block 128) into
    3-bank PSUM page tiles (pool of 2); exp runs on ScalarE as ONE
    activation per page (up to 3 key blocks = 1536 cols, multi-bank
    read) to amortize the ~0.35us fixed cost per activation. The 4
    ragged diagonal blocks are packed into 2.5 banks of one page and
    exp'd by a single 1280-col activation -- no garbage columns.
  - P^T tiles feed PV matmuls as stationary weights; V carries an
    appended ones-column so the softmax denominator accumulates in the
    same PSUM tile (column 128). The 4 PV output slabs (129 cols each)
    are packed 2-per-bank into 2 PSUM banks via first-write start /
    last-write stop flags.
  - Items are phase-shifted: item n's PV matmuls are woven between item
    n+1's QK groups as in-order filler, so page-recycle and exp-latency
    waits never idle the PE (which would also drop its p-state clock).
  - out = PV / denom via one batched DVE reciprocal + one broadcast
    multiply per item.
"""

import numpy as np
import ml_dtypes

import concourse.bass as bass
import concourse.tile as tile
from concourse import bacc, mybir
from concourse.bass import ts
from concourse.bass_utils import run_bass_kernel_spmd
from concourse.masks import make_upper_triangular

B = 2
S = 2048
H = 32
HK = 8
D = 128
NCORES = 8
GH = H // NCORES  # query heads per core (= group size here)
SCALE = 0.08838834764831845  # 1/sqrt(128)

F32 = mybir.dt.float32
BF16 = mybir.dt.bfloat16

NQB = S // 128  # 16 query/key blocks of 128
NSB = 4  # query superblocks of 512
PAGE = 3  # PSUM page size in banks; two pages rotate


def build_nc() -> bass.Bass:
    nc = bacc.Bacc(
        "TRN2", target_bir_lowering=False, debug=False, num_devices=NCORES
    )
    # host-staged layouts: qT [b, g, d, s], kT [b, d, s], v [b, s, d]
    q_d = nc.declare_dram_parameter("query", [B, GH, D, S], BF16, isOutput=False)
    k_d = nc.declare_dram_parameter("key", [B, D, S], BF16, isOutput=False)
    v_d = nc.declare_dram_parameter("value", [B, S, D], BF16, isOutput=False)
    o_d = nc.declare_dram_parameter("out", [B, S, GH, D], F32, isOutput=True)

    with tile.TileContext(nc) as tc:
        with (
            tc.tile_pool(name="consts", bufs=1) as consts,
            tc.tile_pool(name="pt", bufs=11) as pt_pool,
            tc.tile_pool(name="ptd", bufs=4) as ptd_pool,
            tc.tile_pool(name="osb", bufs=3) as osb_pool,
            tc.tile_pool(name="small", bufs=8) as small_pool,
            tc.tile_pool(name="psum", bufs=1, space="PSUM") as psum_pool,
        ):
            # mask[k, q] = 1 where q >= k (keep), 0 above -> kills k > q.
            mask = consts.tile([128, 128], BF16)
            make_upper_triangular(nc, mask, val=1.0, diag=True)

            kt_all = consts.tile([128, B, S], BF16)  # [d, b, k]
            qt_all = consts.tile([128, B * GH, S], BF16)  # [d, inst, q]
            v_ext = consts.tile([128, B, NQB, 132], BF16)  # [k, b, kblk, d+1]

            # PSUM: two 3-bank QK page tiles (pool) + 2-bank packed PV acc.
            # Each page is its own pool tile so dependency tracking works
            # at page granularity: QK of group g+1 must not serialize
            # behind the exp of group g.
            po = psum_pool.tile([128, 2, 512], F32)

            def next_page():
                return psum_pool.tile(
                    [128, PAGE, 512], F32, tag="page", bufs=2, name="pg"
                )

            # ---- startup loads, first-needed first, in 512-col chunks so
            # item (0, sq) unblocks as soon as its kt/qt slices land.
            nc.vector.memset(v_ext[:, :, :, 128:129], 1.0)
            for c0 in range(0, 1024, 512):
                nc.sync.dma_start(
                    out=kt_all[:, 0, c0 : c0 + 512], in_=k_d[0, :, c0 : c0 + 512]
                )
                nc.sync.dma_start(
                    out=qt_all[:, 0, c0 : c0 + 512], in_=q_d[0, 0, :, c0 : c0 + 512]
                )
            nc.sync.dma_start(
                out=v_ext[:, 0, 0:8, 0:128],
                in_=v_d[0, 0:1024, :].rearrange("(n p) d -> p n d", p=128),
            )
            for c0 in range(1024, 2048, 512):
                nc.sync.dma_start(
                    out=kt_all[:, 0, c0 : c0 + 512], in_=k_d[0, :, c0 : c0 + 512]
                )
                nc.sync.dma_start(
                    out=qt_all[:, 0, c0 : c0 + 512], in_=q_d[0, 0, :, c0 : c0 + 512]
                )
            nc.sync.dma_start(
                out=v_ext[:, 0, 8:16, 0:128],
                in_=v_d[0, 1024:2048, :].rearrange("(n p) d -> p n d", p=128),
            )
            nc.sync.dma_start(out=qt_all[:, 1, :], in_=q_d[0, 1, :, :])
            nc.sync.dma_start(out=kt_all[:, 1, :], in_=k_d[1, :, :])
            nc.sync.dma_start(
                out=v_ext[:, 1, :, 0:128],
                in_=v_d[1, :, :].rearrange("(n p) d -> p n d", p=128),
            )

            # pending = (pv_closures, finish_closure, cursor) of prev item
            pending = [None]

            def emit_pending_chunk(frac_done):
                if pending[0] is None:
                    return
                pvs, _fin, cursor = pending[0]
                tgt = int(len(pvs) * frac_done + 0.5)
                while cursor[0] < tgt:
                    pvs[cursor[0]]()
                    cursor[0] += 1

            def finish_pending():
                if pending[0] is None:
                    return
                pvs, fin, cursor = pending[0]
                while cursor[0] < len(pvs):
                    pvs[cursor[0]]()
                    cursor[0] += 1
                fin()
                pending[0] = None

            # sq0 items have a single tiny QK group; their diag QK+exp is
            # hoisted into the preceding (sq3) item's emission so ScalarE
            # never waits out the instance boundary.
            hoisted = {}

            def emit_diag(inst, sq):
                """QK + exp + masks for the 4 ragged diagonal blocks,
                packed into 2.5 banks of one page. Flat col mapping:
                ki+0 -> 0:512, ki+1 -> 512:896, ki+3 -> 896:1024,
                ki+2 -> 1024:1280. Returns the exp'd [128, 1280] tile."""
                b, _g = divmod(inst, GH)
                q0 = 512 * sq
                kd = 4 * sq
                pg = next_page()
                nc.tensor.matmul(
                    pg[:, 0, :],
                    lhsT=kt_all[:, b, ts(kd, 128)],
                    rhs=qt_all[:, inst, q0 : q0 + 512],
                    start=True, stop=True,
                )
                nc.tensor.matmul(
                    pg[:, 1, 0:384],
                    lhsT=kt_all[:, b, ts(kd + 1, 128)],
                    rhs=qt_all[:, inst, q0 + 128 : q0 + 512],
                    start=True, stop=False,
                )
                nc.tensor.matmul(
                    pg[:, 1, 384:512],
                    lhsT=kt_all[:, b, ts(kd + 3, 128)],
                    rhs=qt_all[:, inst, q0 + 384 : q0 + 512],
                    start=False, stop=True,
                )
                nc.tensor.matmul(
                    pg[:, 2, 0:256],
                    lhsT=kt_all[:, b, ts(kd + 2, 128)],
                    rhs=qt_all[:, inst, q0 + 256 : q0 + 512],
                    start=True, stop=True,
                )
                ptd = ptd_pool.tile([128, 1280], BF16)
                pgb = pg[:, :, :]
                diag_in = bass.AP(
                    pgb.tensor, pgb.offset, [list(pgb.ap[0]), [1, 1280]]
                )
                nc.scalar.activation(
                    ptd, diag_in, mybir.ActivationFunctionType.Exp, scale=SCALE
                )
                for c0 in (0, 512, 896, 1024):
                    nc.vector.tensor_mul(
                        ptd[:, c0 : c0 + 128], ptd[:, c0 : c0 + 128], mask
                    )
                return ptd

            def phase_item(inst, sq):
                b, g = divmod(inst, GH)
                q0 = 512 * sq
                noff = 4 * sq
                kd = 4 * sq
                ngroups = (noff + PAGE - 1) // PAGE + 1

                totals = [noff + 1 + j for j in range(4)]
                bank_tot = [totals[0] + totals[1], totals[2] + totals[3]]
                bank_cnt = [0, 0]
                pvs = []

                def defer_pv(pt_ap, kk, j):
                    def run(pt_ap=pt_ap, kk=kk, j=j):
                        bk = j // 2
                        off = (j % 2) * 256
                        bank_cnt[bk] += 1
                        nc.tensor.matmul(
                            po[:, bk, off : off + 129],
                            lhsT=pt_ap,
                            rhs=v_ext[:, b, kk, 0:129],
                            start=(bank_cnt[bk] == 1),
                            stop=(bank_cnt[bk] == bank_tot[bk]),
                        )
                    pvs.append(run)

                gi = 0
                ki = 0
                while ki < noff:
                    n = min(PAGE, noff - ki)
                    pg = next_page()
                    for t in range(n):
                        nc.tensor.matmul(
                            pg[:, t, :],
                            lhsT=kt_all[:, b, ts(ki + t, 128)],
                            rhs=qt_all[:, inst, q0 : q0 + 512],
                            start=True,
                            stop=True,
                        )
                    pt = pt_pool.tile([128, PAGE, 512], BF16)
                    nc.scalar.activation(
                        pt[:, 0:n, :],
                        pg[:, 0:n, :],
                        mybir.ActivationFunctionType.Exp,
                        scale=SCALE,
                    )
                    for t in range(n):
                        for j in range(4):
                            defer_pv(pt[:, t, ts(j, 128)], ki + t, j)
                    ki += n
                    gi += 1
                    emit_pending_chunk(gi / ngroups)

                # --- diagonal group (possibly pre-emitted by the hoist)
                if (inst, sq) in hoisted:
                    ptd = hoisted.pop((inst, sq))
                else:
                    ptd = emit_diag(inst, sq)

                for j in range(4):
                    defer_pv(ptd[:, 128 * j : 128 * (j + 1)], kd, j)
                for j in range(1, 4):
                    defer_pv(ptd[:, 512 + 128 * (j - 1) : 512 + 128 * j], kd + 1, j)
                for j in range(2, 4):
                    defer_pv(ptd[:, 1024 + 128 * (j - 2) : 1024 + 128 * (j - 1)], kd + 2, j)
                defer_pv(ptd[:, 896:1024], kd + 3, 3)

                # --- hoist the NEXT instance's sq0 diag QK+exp here, so
                # at the instance boundary ScalarE rolls straight into it
                if sq == NSB - 1 and inst + 1 < B * GH:
                    hoisted[(inst + 1, 0)] = emit_diag(inst + 1, 0)

                # --- finish previous item (its remaining PV + normalize)
                finish_pending()

                def finish():
                    # batched normalize: slab j sits at po offset j*256,
                    # denominator at col 128 of each slab. One reciprocal
                    # over all 4 denominators + one broadcast multiply.
                    o_sb = osb_pool.tile([128, 4, 128], F32)
                    recip = small_pool.tile([128, 4], F32)
                    base = po[:, :, :]
                    pp = list(base.ap[0])
                    den_ap = bass.AP(
                        base.tensor, base.offset + 128, [pp, [256, 4], [1, 1]]
                    )
                    pv_ap = bass.AP(
                        base.tensor, base.offset, [pp, [256, 4], [1, 128]]
                    )
                    nc.vector.reciprocal(recip, den_ap)
                    rb = recip[:, :]
                    rb_b = bass.AP(
                        rb.tensor, rb.offset, [list(rb.ap[0]), [1, 4], [0, 128]]
                    )
                    ob = o_sb[:, :, :]
                    ob3 = bass.AP(
                        ob.tensor, ob.offset, [list(ob.ap[0]), [128, 4], [1, 128]]
                    )
                    nc.vector.tensor_mul(ob3, pv_ap, rb_b)
                    nc.sync.dma_start(
                        out=o_d[b, q0 : q0 + 512, g, :].rearrange(
                            "(n p) d -> p n d", p=128
                        ),
                        in_=o_sb,
                    )

                pending[0] = (pvs, finish, [0])

            # qt for inst is loaded two instances ahead so QK never waits.
            # The last instance runs its superblocks big-to-small so the
            # pipeline tail drains the 10-matmul sq0 item, not sq3's 58.
            for inst in range(B * GH):
                if inst + 2 < B * GH:
                    bn, gn = divmod(inst + 2, GH)
                    nc.sync.dma_start(
                        out=qt_all[:, inst + 2, :], in_=q_d[bn, gn, :, :]
                    )
                for sq in range(NSB):
                    phase_item(inst, sq)
            finish_pending()

    nc.finalize()
    return nc


def _to_bf16(x):
    return np.asarray(x, dtype=np.float32).astype(ml_dtypes.bfloat16)


def make_in_maps(query, key, value):
    # host-side staging: bf16 cast + [d, s] transposes for Q and K
    qb = _to_bf16(query)  # [B, S, H, D]
    kb = _to_bf16(key)  # [B, S, HK, D]
    vb = _to_bf16(value)
    in_maps = []
    for c in range(NCORES):
        qt = np.ascontiguousarray(
            qb[:, :, GH * c : GH * (c + 1), :].transpose(0, 2, 3, 1)
        )  # [B, GH, D, S]
        kt = np.ascontiguousarray(kb[:, :, c, :].transpose(0, 2, 1))  # [B, D, S]
        vv = np.ascontiguousarray(vb[:, :, c, :])  # [B, S, D]
        in_maps.append({"query": qt, "key": kt, "value": vv})
    return in_maps


def kernel(query, key, value):
    nc = build_nc()
    res = run_bass_kernel_spmd(
        nc, make_in_maps(query, key, value), core_ids=list(range(NCORES))
    )
    outs = [np.asarray(res.results[c]["out"]) for c in range(NCORES)]
    return np.concatenate(outs, axis=2).astype(np.float32)


if __name__ == "__main__":
    rng = np.random.default_rng(0)
    q = rng.standard_normal((B, S, H, D), dtype=np.float32)
    k = rng.standard_normal((B, S, HK, D), dtype=np.float32)
    v = rng.standard_normal((B, S, HK, D), dtype=np.float32)
    out = kernel(q, k, v)
    print("out", out.shape, out.dtype, float(np.abs(out).max()))
